# revision 1
# baseline (speedup 1.0000x reference)
"""MultiHeadAttention TRN2 kernel: 8-way (batch x head-half) sharding.

Core c handles batch b=c//2, heads g*8..g*8+8 where g=c%2.

Per core: Q^T/K^T projections (lhsT=W-slice, rhs=X^T with X^T pre-transposed
on the host), V in natural layout with a fused ones-column (softmax
denominators fall out of the P@V matmul), scores computed transposed (keys on
partition, so the key mask folds into the per-partition ACT bias of the exp),
PV matmul -> ctx^T, normalization fused into the PSUM eviction, partial FC
(row-slice of Wfc). The two half-head partials per batch are summed on the
host while unsharding.

Masked keys contribute exactly zero attention weight (exp of -1e30 underflows
to 0 in the reference), so the host compacts K/V inputs to the unmasked keys
(padded to a fixed LK with a -30000 bias so padding also exps to exactly 0).
With a ~Bernoulli(0.5) mask this halves the attention/softmax work. If a mask
ever leaves more than LK keys unmasked (probability ~1e-8 for the spec's
fill), kernel() falls back to a host computation.

All matmuls run in float32r (full PE rate at N>=256, ~1e-4 rel accuracy).
The exp pass on the Scalar engine (1 elem/lane/cycle, dtype-independent) is
the critical resource; emission interleaves the Q-projection tail and the FC
into the attention stream so PE work hides under the ACT-bound phase, and a
single shared PSUM pool avoids phase-boundary serialization.
"""

import numpy as np

import concourse.mybir as mybir
import concourse.tile as tile
from concourse import bacc
from concourse.bass import ts

F32 = mybir.dt.float32
F32R = mybir.dt.float32r
AF = mybir.ActivationFunctionType

BS, L, D = 4, 2048, 1024
NCORES = 8
H = 8                 # heads per core
DK = 64
HD = H * DK           # 512: head dims per core
LK = 1152             # padded compacted-key length (9 chunks of 128)
NEGB = -30000.0       # masked/padded-key bias (exp underflows to exactly 0)
SCALE = 1.0 / 8.0     # 1/sqrt(DK)


def _build(lk):
    kck = lk // 128           # key chunks
    nkv = (lk + 511) // 512   # 512-wide column blocks of the compacted keys
    nc = bacc.Bacc()
    xt = nc.declare_dram_parameter("xt", [8, 128, L], F32R, isOutput=False)
    xkv = nc.declare_dram_parameter("xkv", [8, 128, lk], F32R, isOutput=False)
    wq = nc.declare_dram_parameter("wq", [8, 128, HD], F32R, isOutput=False)
    wk = nc.declare_dram_parameter("wk", [8, 128, HD], F32R, isOutput=False)
    wv = nc.declare_dram_parameter("wv", [8, 128, HD], F32R, isOutput=False)
    wfc = nc.declare_dram_parameter("wfc", [4, 128, D], F32R, isOutput=False)
    bq = nc.declare_dram_parameter("bq", [4, 128, 1], F32, isOutput=False)
    bk = nc.declare_dram_parameter("bk", [4, 128, 1], F32, isOutput=False)
    bvr = nc.declare_dram_parameter("bvr", [1, HD], F32R, isOutput=False)
    bfch = nc.declare_dram_parameter("bfch", [8, 128, 1], F32, isOutput=False)
    mb = nc.declare_dram_parameter("mb", [128, kck], F32, isOutput=False)
    outp = nc.declare_dram_parameter("out", [8, 128, L], F32, isOutput=True)

    with tile.TileContext(nc) as tc:
        with tc.tile_pool(name="const", bufs=1) as pc, \
             tc.tile_pool(name="qt", bufs=4) as p_qt, \
             tc.tile_pool(name="kt", bufs=4) as p_kt, \
             tc.tile_pool(name="v", bufs=kck) as p_v, \
             tc.tile_pool(name="ctx", bufs=4) as p_ctx, \
             tc.tile_pool(name="pt", bufs=4, side="right") as p_pt, \
             tc.tile_pool(name="smallB", bufs=2, side="right") as p_sm, \
             tc.tile_pool(name="ps", bufs=2, space="PSUM") as PS:
            # constants
            ones_f = pc.tile([1, 128], F32)
            nc.vector.memset(ones_f[:], 1.0)
            ones_r = pc.tile([1, 128], F32R)
            nc.vector.tensor_copy(ones_r[:], ones_f[:])
            onesv = pc.tile([128, 8, 1], F32)
            nc.vector.memset(onesv[:], 1.0)
            mb_sb = pc.tile([128, kck], F32)
            nc.sync.dma_start(out=mb_sb[:], in_=mb[:])
            bv_sb = pc.tile([1, HD], F32R)
            nc.sync.dma_start(out=bv_sb[:], in_=bvr[:])

            qt_t = [p_qt.tile([128, L], F32R, tag="qt", name=f"qt{i}")
                    for i in range(4)]
            kt_t = [p_kt.tile([128, lk], F32R, tag="kt", name=f"kt{i}")
                    for i in range(4)]
            v_t = [p_v.tile([128, 8, 65], F32R, tag="v", name=f"v{i}")
                   for i in range(kck)]
            ctx_t = [p_ctx.tile([128, L], F32R, tag="ctx", name=f"ctx{i}")
                     for i in range(4)]

            def attn_head(q, h):
                q0 = q * 1024
                th, oh = h // 2, (h % 2) * 64
                cps = [PS.tile([65, 512], F32, tag="ctxp", name=f"c{half}")
                       for half in range(2)]
                for kc in range(kck):
                    sps = PS.tile([128, 1024], F32, tag="s", name="s")
                    for half in range(2):
                        nc.tensor.matmul(
                            sps[:, half * 512:(half + 1) * 512],
                            kt_t[th][oh:oh + 64, ts(kc, 128)],
                            qt_t[th][oh:oh + 64,
                                     q0 + half * 512:q0 + (half + 1) * 512],
                            start=True, stop=True)
                    pt = p_pt.tile([128, 1024], F32R, tag="pt", name="pt")
                    nc.scalar.activation(pt[:], sps[:], AF.Exp,
                                         bias=mb_sb[:, kc:kc + 1], scale=SCALE)
                    st, sp = (kc == 0), (kc == kck - 1)
                    for half in range(2):
                        nc.tensor.matmul(cps[half][:], v_t[kc][:, h, :],
                                         pt[:, half * 512:(half + 1) * 512],
                                         start=st, stop=sp)
                for half in range(2):
                    den = p_sm.tile([1, 512], F32R, tag="den", name="den")
                    nc.vector.tensor_copy(den[:], cps[half][64:65, :])
                    rbps = PS.tile([64, 512], F32, tag="mm", name="rbps")
                    nc.tensor.matmul(rbps[:], ones_r[:, 0:64], den[:],
                                     start=True, stop=True)
                    rbs = p_sm.tile([64, 512], F32, tag="rbs", name="rbs")
                    nc.vector.reciprocal(rbs[:], rbps[:])
                    nc.vector.tensor_mul(
                        ctx_t[th][oh:oh + 64,
                                  q0 + half * 512:q0 + (half + 1) * 512],
                        cps[half][0:64, :], rbs[:])

            # ---------------- Phase A + attention(q=0) ----------------
            with tc.tile_pool(name="wres", bufs=24) as p_w, \
                 tc.tile_pool(name="xn", bufs=9) as p_xn, \
                 tc.tile_pool(name="biasA", bufs=8) as p_b:
                wk_r, wq_r, wv_r = [], [], []
                for k in range(8):
                    ck = p_w.tile([128, HD], F32R, tag="w", name=f"wk{k}")
                    nc.sync.dma_start(out=ck[:], in_=wk[k])
                    wk_r.append(ck)
                for k in range(8):
                    cv = p_w.tile([128, HD], F32R, tag="w", name=f"wv{k}")
                    nc.sync.dma_start(out=cv[:], in_=wv[k])
                    wv_r.append(cv)
                for k in range(8):
                    cq = p_w.tile([128, HD], F32R, tag="w", name=f"wq{k}")
                    nc.sync.dma_start(out=cq[:], in_=wq[k])
                    wq_r.append(cq)
                bq_t, bk_t = [], []
                for t in range(4):
                    bt = p_b.tile([128, 1], F32, tag="b", name=f"bq{t}")
                    nc.sync.dma_start(out=bt[:], in_=bq[t])
                    bq_t.append(bt)
                    bt = p_b.tile([128, 1], F32, tag="b", name=f"bk{t}")
                    nc.sync.dma_start(out=bt[:], in_=bk[t])
                    bk_t.append(bt)

                # K^T and V from streamed Xkv^T column blocks
                for n in range(nkv):
                    c0 = n * 512
                    w = min(512, lk - c0)
                    xkn = []
                    for k in range(8):
                        t_ = p_xn.tile([128, 512], F32R, tag="xn", name=f"xkn{k}")
                        nc.sync.dma_start(out=t_[:, :w], in_=xkv[k][:, c0:c0 + w])
                        xkn.append(t_)
                    for t in range(4):
                        ps = PS.tile([128, 512], F32, tag="mm", name="psk")
                        for k in range(8):
                            nc.tensor.matmul(ps[:, :w], wk_r[k][:, ts(t, 128)],
                                             xkn[k][:, :w],
                                             start=(k == 0), stop=(k == 7))
                        nc.vector.tensor_scalar_add(
                            kt_t[t][:, c0:c0 + w], ps[:, :w], bk_t[t][:])
                    for mi in range(w // 128):
                        m = n * 4 + mi
                        ps = PS.tile([128, 512], F32, tag="mm", name="psv")
                        for k in range(8):
                            nc.tensor.matmul(ps[:], xkn[k][:, ts(mi, 128)],
                                             wv_r[k][:],
                                             start=(k == 0), stop=False)
                        nc.tensor.matmul(ps[:], ones_r[:, :128], bv_sb[:],
                                         start=False, stop=True)
                        nc.vector.tensor_copy(
                            v_t[m][:, :, 0:64],
                            ps[:].rearrange("p (h d) -> p h d", h=8))
                        nc.vector.tensor_copy(v_t[m][:, :, 64:65], onesv[:])

                def q_block(n):
                    xtn = []
                    for k in range(8):
                        t_ = p_xn.tile([128, 512], F32R, tag="xn", name=f"xtn{k}")
                        nc.sync.dma_start(out=t_[:], in_=xt[k][:, ts(n, 512)])
                        xtn.append(t_)
                    for t in range(4):
                        ps = PS.tile([128, 512], F32, tag="mm", name="psq")
                        for k in range(8):
                            nc.tensor.matmul(ps[:], wq_r[k][:, ts(t, 128)],
                                             xtn[k][:],
                                             start=(k == 0), stop=(k == 7))
                        nc.vector.tensor_scalar_add(qt_t[t][:, ts(n, 512)],
                                                    ps[:], bq_t[t][:])

                q_block(0)
                q_block(1)

                # attention on query block 0; the remaining Q projection is
                # interleaved so its matmuls hide under the ACT-bound stream
                for h in range(H):
                    attn_head(0, h)
                    if h == 0:
                        q_block(2)
                    elif h == 1:
                        q_block(3)

            # ---------------- attention(q=1) + FC ----------------
            with tc.tile_pool(name="wfc", bufs=4) as p_wfc, \
                 tc.tile_pool(name="biasC", bufs=8) as p_bc, \
                 tc.tile_pool(name="ev", bufs=4) as p_ev:
                wfc_c = []
                for k in range(4):
                    cf = p_wfc.tile([128, D], F32R, tag="wfc")
                    nc.sync.dma_start(out=cf[:], in_=wfc[k])
                    wfc_c.append(cf)
                bfc_t = []
                for m in range(8):
                    bt = p_bc.tile([128, 1], F32, tag="bc", name=f"bfc{m}")
                    nc.sync.dma_start(out=bt[:], in_=bfch[m])
                    bfc_t.append(bt)

                def fc_nblock(n):
                    for m in range(8):
                        ps = PS.tile([128, 512], F32, tag="mm", name="f")
                        for k in range(4):
                            nc.tensor.matmul(ps[:], wfc_c[k][:, ts(m, 128)],
                                             ctx_t[k][:, ts(n, 512)],
                                             start=(k == 0), stop=(k == 3))
                        ev = p_ev.tile([128, 512], F32, tag="ev")
                        nc.vector.tensor_scalar_add(ev[:], ps[:], bfc_t[m][:])
                        nc.sync.dma_start(out=outp[m][:, ts(n, 512)], in_=ev[:])

                for h in range(H):
                    attn_head(1, h)
                    if h == 0:
                        fc_nblock(0)
                    elif h == 2:
                        fc_nblock(1)
                fc_nblock(2)
                fc_nblock(3)

    nc.finalize()
    return nc


class _Runner:
    """Compile-once wrapper around the run_bass_via_pjrt shard_map path."""

    def __init__(self, nc):
        import jax
        from jax.sharding import Mesh, PartitionSpec

        from concourse import bass2jax, mybir as mb

        try:
            from jax.experimental.shard_map import shard_map
        except ImportError:
            from jax.shard_map import shard_map

        bass2jax.install_neuronx_cc_hook()
        self._nc = nc
        partition_name = (nc.partition_id_tensor.name
                          if nc.partition_id_tensor else None)
        in_names, out_names, out_avals = [], [], []
        self._zero_shapes = []
        for alloc in nc.m.functions[0].allocations:
            if not isinstance(alloc, mb.MemoryLocationSet):
                continue
            name = alloc.memorylocations[0].name
            if alloc.kind == "ExternalInput":
                if name != partition_name:
                    in_names.append(name)
            elif alloc.kind == "ExternalOutput":
                out_names.append(name)
                shape = tuple(alloc.tensor_shape)
                dtype = mb.dt.np(alloc.dtype)
                out_avals.append(jax.core.ShapedArray(shape, dtype))
                self._zero_shapes.append((shape, dtype))
        self._n_params = len(in_names)
        n_outs = len(out_avals)
        self._in_names = list(in_names)
        self._out_names = list(out_names)
        self._out_avals = out_avals
        all_in = in_names + out_names
        if partition_name is not None:
            all_in.append(partition_name)

        def _body(*args):
            operands = list(args)
            if partition_name is not None:
                operands.append(bass2jax.partition_id_tensor())
            return tuple(bass2jax._bass_exec_p.bind(
                *operands,
                out_avals=tuple(out_avals),
                in_names=tuple(all_in),
                out_names=tuple(out_names),
                lowering_input_output_aliases=(),
                sim_require_finite=True,
                sim_require_nnan=True,
                nc=nc,
            ))

        devices = jax.devices()[:NCORES]
        mesh = Mesh(np.asarray(devices), ("core",))
        self.mesh = mesh
        nin = self._n_params + n_outs
        self._sharded = jax.jit(
            shard_map(_body, mesh=mesh,
                      in_specs=(PartitionSpec("core"),) * nin,
                      out_specs=(PartitionSpec("core"),) * n_outs,
                      check_rep=False),
            donate_argnums=tuple(range(self._n_params, nin)),
            keep_unused=True,
        )

    def run(self, in_maps):
        import jax
        concat_in = [
            np.concatenate([np.asarray(in_maps[c][name])
                            for c in range(NCORES)], axis=0)
            for name in self._in_names
        ]
        concat_zeros = [np.zeros((NCORES * s[0], *s[1:]), d)
                        for s, d in self._zero_shapes]
        out_arrs = self._sharded(*concat_in, *concat_zeros)
        jax.block_until_ready(out_arrs)
        return [
            {name: np.asarray(out_arrs[i]).reshape(
                NCORES, *self._out_avals[i].shape)[c]
             for i, name in enumerate(self._out_names)}
            for c in range(NCORES)
        ]


_RUNNERS = {}


def _get_runner(lk):
    if lk not in _RUNNERS:
        _RUNNERS[lk] = _Runner(_build(lk))
    return _RUNNERS[lk]


def _prep_in_maps(x, mask, Wq, bq, Wk, bk, Wv, bv, Wfc, bfc):
    """Shard + lay out the full inputs for the 8 cores.

    Returns (in_maps, lk) or (None, None) if the mask leaves more than LK
    keys unmasked in some batch (host fallback).
    """
    keep = [np.nonzero(mask[b] == 0)[0] for b in range(BS)]
    if max(len(kp) for kp in keep) > LK or min(len(kp) for kp in keep) == 0:
        # too many unmasked keys for the compiled shape, or a fully-masked
        # batch (reference degenerates to uniform attention there)
        return None, None
    lk = LK

    in_maps = []
    for c in range(NCORES):
        b, g = c // 2, c % 2
        sl = slice(g * HD, (g + 1) * HD)
        kp = keep[b]
        xkv_b = np.zeros((lk, D), np.float32)
        xkv_b[:len(kp)] = x[b][kp]
        biask = np.where(np.arange(lk) < len(kp), 0.0, NEGB).astype(np.float32)
        in_maps.append({
            "xt": np.ascontiguousarray(x[b].T).reshape(8, 128, L),
            "xkv": np.ascontiguousarray(xkv_b.T).reshape(8, 128, lk),
            "wq": np.ascontiguousarray(Wq[:, sl]).reshape(8, 128, HD),
            "wk": np.ascontiguousarray(Wk[:, sl]).reshape(8, 128, HD),
            "wv": np.ascontiguousarray(Wv[:, sl]).reshape(8, 128, HD),
            "wfc": np.ascontiguousarray(Wfc[sl, :]).reshape(4, 128, D),
            "bq": np.ascontiguousarray(bq[sl]).reshape(4, 128, 1),
            "bk": np.ascontiguousarray(bk[sl]).reshape(4, 128, 1),
            "bvr": np.ascontiguousarray(bv[sl]).reshape(1, HD),
            "bfch": np.ascontiguousarray(bfc * 0.5).reshape(8, 128, 1),
            "mb": np.ascontiguousarray(biask.reshape(lk // 128, 128).T),
        })
    return in_maps, lk


def _host_reference(x, mask, Wq, bq, Wk, bk, Wv, bv, Wfc, bfc):
    """Numpy fallback, bit-compatible with the reference semantics."""
    out = np.empty((BS, L, D), np.float32)
    for b in range(BS):
        q = (x[b] @ Wq + bq).reshape(L, 16, DK).transpose(1, 0, 2)
        k = (x[b] @ Wk + bk).reshape(L, 16, DK).transpose(1, 0, 2)
        v = (x[b] @ Wv + bv).reshape(L, 16, DK).transpose(1, 0, 2)
        s = np.einsum("hqd,hkd->hqk", q, k) * SCALE
        m = mask[b].astype(np.float32)[None, None, :]
        s = s * (1.0 - m) + m * (-1e30)
        s = s - s.max(axis=-1, keepdims=True)
        p = np.exp(s)
        p /= p.sum(axis=-1, keepdims=True)
        o = np.einsum("hqk,hkd->hqd", p, v).transpose(1, 0, 2).reshape(L, D)
        out[b] = o @ Wfc + bfc
    return out


def kernel(x, mask, Wq, bq, Wk, bk, Wv, bv, Wfc, bfc, **_unused):
    x = np.asarray(x, np.float32)
    mask = np.asarray(mask)
    Wq, bq = np.asarray(Wq, np.float32), np.asarray(bq, np.float32)
    Wk, bk = np.asarray(Wk, np.float32), np.asarray(bk, np.float32)
    Wv, bv = np.asarray(Wv, np.float32), np.asarray(bv, np.float32)
    Wfc, bfc = np.asarray(Wfc, np.float32), np.asarray(bfc, np.float32)

    in_maps, lk = _prep_in_maps(x, mask, Wq, bq, Wk, bk, Wv, bv, Wfc, bfc)
    if in_maps is None:
        return _host_reference(x, mask, Wq, bq, Wk, bk, Wv, bv, Wfc, bfc)
    results = _get_runner(lk).run(in_maps)

    out = np.empty((BS, L, D), np.float32)
    for b in range(BS):
        p0 = results[2 * b]["out"].reshape(D, L)
        p1 = results[2 * b + 1]["out"].reshape(D, L)
        out[b] = (p0 + p1).T
    return out

